# revision 10
# baseline (speedup 1.0000x reference)
"""DCM (dynamic conv module) Trainium2 kernel — single-pass x streaming.

Reference computation (per sample b, channel c):
  f[b,c,3,3]  = adaptive_avg_pool2d(x[b,c], 3)        # dynamic depthwise filter
  out[b,c]    = depthwise_conv3x3(x[b,c], f[b,c])     # zero padding 1
  y           = relu(batchnorm_train(out, gamma, beta))  # batch stats over (B,H,W)

Sharding: data-parallel over batch B=16 across 8 cores (2 samples/core).
Sync-BN via a [C,2] AllReduce of per-channel (sum, sumsq).

Per-core layout: channels C=128 on partitions, free dim = H*W per sample.
x is streamed from HBM ONCE per sample as 8 halo chunks (18 rows each)
into a 10-deep rolling SBUF pool; the pooling pass reads the chunk
interiors and the conv taps read the same tiles (tiles are declared
float32r so the PE runs at 1 cyc/row; DMA and DVE access them through
bitcast-f32 views).  Conv = 7 shifted taps as diag(f_tap) matmuls in
PSUM (the two dj==0 taps are added by DVE scalar_tensor_tensor directly
into PSUM, offloading the PE).  Horizontal zero-padding is handled by
letting taps wrap across row ends and subtracting the wrapped term on
the two edge columns.  Conv output is stored bf16-resident in SBUF
(halves residency + output traffic); BN sums/sumsq accumulate from the
f32 PSUM tiles so the statistics stay exact.  After the stats
AllReduce, BN+ReLU is applied in-place (ACT/DVE split) and the y tiles
are DMA'd out as bf16 (converted to f32 on the host).
"""

import numpy as np

# ---------------------------------------------------------------- constants
B, C, H, W = 16, 128, 128, 128
N_CORES = 8
BL = B // N_CORES          # samples per core
HW = H * W                 # 16384 free elems per plane
FS = 3
BN_EPS = 1e-5

ROWS = 16                  # output rows per psum tile
NCHUNK = H // ROWS         # 8 chunks per plane
TILE_F = ROWS * W          # 2048 free elems per psum tile
XT_F = (ROWS + 2) * W + 2  # x chunk with halo rows + 1 elem pad each end
NPS = NCHUNK * BL          # psum/out tiles per core

# adaptive_avg_pool2d(3) bin boundaries (PyTorch convention)
SH = [(i * H) // FS for i in range(FS)]
EH = [-((-(i + 1) * H) // FS) for i in range(FS)]
SW = [(i * W) // FS for i in range(FS)]
EW = [-((-(i + 1) * W) // FS) for i in range(FS)]

TAPS = [(di, dj) for di in (-1, 0, 1) for dj in (-1, 0, 1)]  # t = 3*(di+1)+(dj+1)
DVE_TAPS = [1, 7]                      # (di=-1,dj=0), (di=+1,dj=0): no wrap fixup
PE_TAPS = [t for t in range(FS * FS) if t not in DVE_TAPS]

MM_N = 512                 # fp32r moving-operand max per matmul
NSL = TILE_F // MM_N       # slices per psum tile

# BN apply split: of every 8 out tiles, this many go to ACT (1 op each),
# the rest to DVE (2 ops each, but bf16 gets 2x DVE throughput).
ACT_BN_PER8 = 5


def _counts_recip():
    cr = np.empty((C, FS * FS), dtype=np.float32)
    for i in range(FS):
        for j in range(FS):
            cr[:, 3 * i + j] = 1.0 / float((EH[i] - SH[i]) * (EW[j] - SW[j]))
    return cr


def build_nc(n_cores: int = N_CORES):
    """Build + compile the per-core Bass program (identical on all cores)."""
    import concourse.bacc as bacc
    import concourse.tile as tile
    from concourse import mybir

    f32 = mybir.dt.float32
    f32r = mybir.dt.float32r
    bf16 = mybir.dt.bfloat16
    AT = mybir.ActivationFunctionType
    OP = mybir.AluOpType
    AX = mybir.AxisListType

    ntot = float(n_cores * BL * HW)   # BN element count per channel

    nc = bacc.Bacc(
        "TRN2",
        target_bir_lowering=False,
        debug=False,
        num_devices=n_cores,
    )

    x_d = nc.dram_tensor("x", [BL, C, HW], f32, kind="ExternalInput").ap()
    gamma_d = nc.dram_tensor("gamma", [C, 1], f32, kind="ExternalInput").ap()
    beta_d = nc.dram_tensor("beta", [C, 1], f32, kind="ExternalInput").ap()
    ident_d = nc.dram_tensor("ident", [C, C], f32, kind="ExternalInput").ap()
    crecip_d = nc.dram_tensor("crecip", [C, FS * FS], f32, kind="ExternalInput").ap()
    y_d = nc.dram_tensor("y", [BL, C, HW], bf16, kind="ExternalOutput").ap()

    with tile.TileContext(nc) as tc:
        with (
            tc.tile_pool(name="singles", bufs=1) as singles,
            tc.tile_pool(name="xpool", bufs=10) as xpool,
            tc.tile_pool(name="outres", bufs=NPS) as outres,
            tc.tile_pool(name="psum", bufs=2, space="PSUM") as psum,
            tc.tile_pool(name="colsp", bufs=2) as colsp,
            tc.tile_pool(name="fpool", bufs=2) as fpool,
            tc.tile_pool(name="diagp", bufs=2 * len(PE_TAPS)) as diagp,
            tc.tile_pool(name="statp", bufs=1) as statp,
            tc.tile_pool(name="dram", bufs=1, space="DRAM") as dram,
        ):
            # ---- constants
            gamma_s = singles.tile([C, 1], f32, tag="gamma")
            nc.sync.dma_start(out=gamma_s[:], in_=gamma_d[:, :])
            beta_s = singles.tile([C, 1], f32, tag="beta")
            nc.sync.dma_start(out=beta_s[:], in_=beta_d[:, :])
            ident_s = singles.tile([C, C], f32, tag="ident")
            nc.sync.dma_start(out=ident_s[:], in_=ident_d[:, :])
            crecip_s = singles.tile([C, FS * FS], f32, tag="crecip")
            nc.sync.dma_start(out=crecip_s[:], in_=crecip_d[:, :])

            sums = statp.tile([C, NPS], f32, tag="sums")
            sumsq = statp.tile([C, NPS], f32, tag="sumsq")

            # Dummy warm-up AllReduce issued at kernel start: absorbs the
            # one-time ncfw ramp so the real stats AllReduce on the critical
            # path is cheaper. Runs concurrently with the first sample load.
            warm = statp.tile([C, 2], f32, tag="warm")
            nc.gpsimd.memset(warm[:], 0.0)
            dw_in = dram.tile([C, 2], f32, tag="dw_in")
            dw_out = dram.tile([C, 2], f32, tag="dw_out")
            nc.sync.dma_start(out=dw_in[:], in_=warm[:])
            nc.gpsimd.collective_compute(
                "AllReduce",
                OP.add,
                replica_groups=[list(range(n_cores))],
                ins=[dw_in[:].opt()],
                outs=[dw_out[:].opt()],
            )

            out_tiles = []
            kpt = 0  # global psum-tile index

            def load_chunk(s, c):
                """DMA one halo chunk of plane s into a fresh f32r x tile.

                Returns (tile, bitcast-f32 view)."""
                xt = xpool.tile([C, XT_F], f32r, tag="xt", name="xt")
                xv = xt[:].bitcast(f32)
                r_lo = c * ROWS - 1
                r_hi = c * ROWS + ROWS + 1
                # 1-elem pads at both ends (read by corner-wrap taps; must be
                # finite so the fixup subtraction cancels exactly).
                nc.vector.memset(xv[:, 0:1], 0.0)
                nc.vector.memset(xv[:, XT_F - 1:XT_F], 0.0)
                if r_lo < 0:
                    nc.vector.memset(xv[:, 1:1 + W], 0.0)
                if r_hi > H:
                    nc.vector.memset(xv[:, 1 + (ROWS + 1) * W:1 + (ROWS + 2) * W], 0.0)
                src_lo = max(r_lo, 0) * W
                src_hi = min(r_hi, H) * W
                dst_lo = 1 + (max(r_lo, 0) - r_lo) * W
                # both DMA sides f32r (bit-identical to f32) so the BIR
                # verifier sees the matmul moving operand produced as f32r
                nc.sync.dma_start(
                    out=xt[:, dst_lo:dst_lo + (src_hi - src_lo)],
                    in_=x_d[s, :, src_lo:src_hi].bitcast(f32r),
                )
                return xt, xv

            # per-sample state
            xts = {}      # (s, c) -> (tile, f32 view)
            colS = {}     # s -> column-sum tile
            fTs = {}      # s -> filter tile [C, 9] f32
            fnegs = {}    # s -> -filter [C, 9] f32
            diags = {}    # s -> {t: diag tile}

            def emit_load(s, c):
                xts[(s, c)] = load_chunk(s, c)

            def emit_pool(s, c):
                if s not in colS:
                    colS[s] = colsp.tile([C, FS, H], f32, tag="colS", name="colS")
                _, xv = xts[(s, c)]
                xiv = xv[:, 1 + W:1 + (ROWS + 1) * W].rearrange(
                    "p (r w) -> p r w", w=W
                )
                for j in range(FS):
                    nc.vector.tensor_reduce(
                        out=colS[s][:, j, c * ROWS:(c + 1) * ROWS],
                        in_=xiv[:, :, SW[j]:EW[j]],
                        axis=AX.X,
                        op=OP.add,
                    )

            def emit_filter(s):
                fT = fpool.tile([C, FS * FS], f32, tag="fT", name="fT")
                for i in range(FS):
                    for j in range(FS):
                        k = 3 * i + j
                        nc.vector.tensor_reduce(
                            out=fT[:, k:k + 1],
                            in_=colS[s][:, j, SH[i]:EH[i]],
                            axis=AX.X,
                            op=OP.add,
                        )
                nc.vector.tensor_mul(fT[:], fT[:], crecip_s[:])
                fneg = fpool.tile([C, FS * FS], f32, tag="fneg", name="fneg")
                nc.vector.tensor_scalar_mul(fneg[:], fT[:], -1.0)
                dgs = {}
                for t in PE_TAPS:
                    dg = diagp.tile([C, C], f32r, tag="diag", name="diag")
                    nc.vector.tensor_scalar_mul(dg[:], ident_s[:], fT[:, t:t + 1])
                    dgs[t] = dg
                fTs[s], fnegs[s], diags[s] = fT, fneg, dgs

            def emit_conv(s, c):
                nonlocal kpt
                xt, xv = xts.pop((s, c))
                fT, fneg, dgs = fTs[s], fnegs[s], diags[s]
                pt = psum.tile([C, TILE_F], f32, tag="pt", name="pt")
                for sl in range(NSL):
                    for it, t in enumerate(PE_TAPS):
                        di, dj = TAPS[t]
                        base = 1 + (di + 1) * W + dj + sl * MM_N
                        nc.tensor.matmul(
                            pt[:, sl * MM_N:(sl + 1) * MM_N],
                            dgs[t][:],
                            xt[:, base:base + MM_N],
                            start=(it == 0),
                            stop=(it == len(PE_TAPS) - 1),
                        )
                # dj==0 taps on DVE, added straight into PSUM
                for t in DVE_TAPS:
                    di, _ = TAPS[t]
                    base = 1 + (di + 1) * W
                    nc.vector.scalar_tensor_tensor(
                        out=pt[:],
                        in0=xv[:, base:base + TILE_F],
                        scalar=fT[:, t:t + 1],
                        in1=pt[:],
                        op0=OP.mult,
                        op1=OP.add,
                    )
                # edge-column fixups: subtract the horizontally wrapped term
                pv = pt[:].rearrange("p (r w) -> p r w", w=W)
                for i, di in enumerate((-1, 0, 1)):
                    # w = 0 read x[h+di, -1] -> wrapped to (h+di-1, W-1)
                    src = xv[:, (di + 1) * W:(di + 1) * W + ROWS * W].rearrange(
                        "p (r w) -> p r w", w=W
                    )[:, :, 0:1]
                    nc.vector.scalar_tensor_tensor(
                        out=pv[:, :, 0:1],
                        in0=src,
                        scalar=fneg[:, 3 * i:3 * i + 1],
                        in1=pv[:, :, 0:1],
                        op0=OP.mult,
                        op1=OP.add,
                    )
                    # w = W-1 read x[h+di, W] -> wrapped to (h+di+1, 0)
                    s0 = (di + 1) * W + 2
                    src = xv[:, s0:s0 + ROWS * W].rearrange(
                        "p (r w) -> p r w", w=W
                    )[:, :, W - 1:W]
                    nc.vector.scalar_tensor_tensor(
                        out=pv[:, :, W - 1:W],
                        in0=src,
                        scalar=fneg[:, 3 * i + 2:3 * i + 3],
                        in1=pv[:, :, W - 1:W],
                        op0=OP.mult,
                        op1=OP.add,
                    )
                # PSUM -> resident bf16 SBUF copy, fused per-channel sum
                ot = outres.tile([C, TILE_F], bf16, tag="ot", name="ot")
                nc.scalar.activation(
                    out=ot[:], in_=pt[:], func=AT.Copy,
                    accum_out=sums[:, kpt:kpt + 1],
                )
                # sum of squares from the exact f32 psum values; squares
                # overwrite the psum tile in place (only accum_out is kept)
                nc.scalar.activation(
                    out=pt[:], in_=pt[:], func=AT.Square,
                    accum_out=sumsq[:, kpt:kpt + 1],
                )
                out_tiles.append((s, c, ot))
                kpt += 1

            # ---------------- software-pipelined emission
            for c in range(NCHUNK):
                emit_load(0, c)
                emit_pool(0, c)
            emit_load(1, 0)
            emit_load(1, 1)
            emit_filter(0)
            for c in range(NCHUNK):
                # interleave sample-1 pooling into sample-0's conv stream so
                # DVE keeps ahead of the PE and the inter-sample bubble is
                # only the (tiny) filter/diag build
                if c >= 1:
                    emit_pool(1, c - 1)
                if c + 2 < NCHUNK:
                    emit_load(1, c + 2)
                emit_conv(0, c)
            emit_pool(1, NCHUNK - 1)
            emit_filter(1)
            for c in range(NCHUNK):
                emit_conv(1, c)

            # ---------------- sync-BN stats AllReduce
            arin = statp.tile([C, 2], f32, tag="arin")
            nc.vector.tensor_reduce(out=arin[:, 0:1], in_=sums[:], axis=AX.X, op=OP.add)
            nc.vector.tensor_reduce(out=arin[:, 1:2], in_=sumsq[:], axis=AX.X, op=OP.add)
            d_in = dram.tile([C, 2], f32, tag="d_in")
            d_out = dram.tile([C, 2], f32, tag="d_out")
            nc.sync.dma_start(out=d_in[:], in_=arin[:])
            nc.gpsimd.collective_compute(
                "AllReduce",
                OP.add,
                replica_groups=[list(range(n_cores))],
                ins=[d_in[:].opt()],
                outs=[d_out[:].opt()],
            )
            aro = statp.tile([C, 2], f32, tag="aro")
            nc.sync.dma_start(out=aro[:], in_=d_out[:])

            # ---------------- BN scale/shift (all [C,1], fp32)
            mean = statp.tile([C, 1], f32, tag="mean")
            nc.vector.tensor_scalar_mul(mean[:], aro[:, 0:1], 1.0 / ntot)
            ex2 = statp.tile([C, 1], f32, tag="ex2")
            nc.vector.tensor_scalar_mul(ex2[:], aro[:, 1:2], 1.0 / ntot)
            var = statp.tile([C, 1], f32, tag="var")
            nc.vector.tensor_mul(var[:], mean[:], mean[:])
            nc.vector.tensor_sub(var[:], ex2[:], var[:])
            veps = statp.tile([C, 1], f32, tag="veps")
            nc.vector.tensor_scalar_add(veps[:], var[:], BN_EPS)
            eps_t = statp.tile([C, 1], f32, tag="eps_t")
            nc.vector.memset(eps_t[:], BN_EPS)
            sd = statp.tile([C, 1], f32, tag="sd")
            nc.scalar.activation(out=sd[:], in_=var[:], func=AT.Sqrt, bias=eps_t[:])
            z = statp.tile([C, 1], f32, tag="z")
            nc.vector.reciprocal(z[:], sd[:])
            # one Newton step: z <- z * (1.5 - 0.5 * veps * z^2)
            nt = statp.tile([C, 1], f32, tag="nt")
            nc.vector.tensor_mul(nt[:], z[:], z[:])
            nc.vector.tensor_mul(nt[:], nt[:], veps[:])
            nc.vector.tensor_scalar(
                out=nt[:], in0=nt[:], scalar1=-0.5, scalar2=1.5,
                op0=OP.mult, op1=OP.add,
            )
            nc.vector.tensor_mul(z[:], z[:], nt[:])
            scale_t = statp.tile([C, 1], f32, tag="scale_t")
            nc.vector.tensor_mul(scale_t[:], gamma_s[:], z[:])
            shift_t = statp.tile([C, 1], f32, tag="shift_t")
            nc.vector.tensor_mul(shift_t[:], mean[:], scale_t[:])
            nc.vector.tensor_sub(shift_t[:], beta_s[:], shift_t[:])

            # ---------------- BN apply + ReLU + writeback (ACT / DVE split)
            for idx, (s, c, ot) in enumerate(out_tiles):
                if idx % 8 < ACT_BN_PER8:
                    nc.scalar.activation(
                        out=ot[:], in_=ot[:], func=AT.Relu,
                        scale=scale_t[:], bias=shift_t[:],
                    )
                else:
                    nc.vector.tensor_scalar(
                        out=ot[:], in0=ot[:],
                        scalar1=scale_t[:], scalar2=shift_t[:],
                        op0=OP.mult, op1=OP.add,
                    )
                    nc.vector.tensor_scalar_max(ot[:], ot[:], 0.0)
                nc.sync.dma_start(
                    out=y_d[s, :, c * TILE_F:(c + 1) * TILE_F], in_=ot[:],
                )

    nc.compile()
    return nc


_NC_CACHE = {}


def _get_nc(n_cores: int = N_CORES):
    if n_cores not in _NC_CACHE:
        _NC_CACHE[n_cores] = build_nc(n_cores)
    return _NC_CACHE[n_cores]


def make_in_maps(x: np.ndarray, gamma: np.ndarray, beta: np.ndarray,
                 n_cores: int = N_CORES):
    x_r = np.ascontiguousarray(
        np.asarray(x, dtype=np.float32).reshape(B, C, HW)
    )
    g = np.ascontiguousarray(np.asarray(gamma, dtype=np.float32).reshape(C, 1))
    b = np.ascontiguousarray(np.asarray(beta, dtype=np.float32).reshape(C, 1))
    ident = np.eye(C, dtype=np.float32)
    crecip = _counts_recip()
    maps = []
    for core in range(n_cores):
        maps.append({
            "x": x_r[core * BL:(core + 1) * BL],
            "gamma": g,
            "beta": b,
            "ident": ident,
            "crecip": crecip,
        })
    return maps


def kernel(x, gamma, beta):
    from concourse import bass_utils

    nc = _get_nc(N_CORES)
    in_maps = make_in_maps(x, gamma, beta, N_CORES)
    res = bass_utils.run_bass_kernel_spmd(nc, in_maps, core_ids=list(range(N_CORES)))
    y = np.concatenate(
        [np.asarray(res.results[c]["y"]) for c in range(N_CORES)], axis=0
    )
    return y.reshape(B, C, H, W).astype(np.float32)
